# revision 13
# baseline (speedup 1.0000x reference)
"""Trainium2 Bass kernel for nn_Attention_79645873537262.

Dense attention with per-head bias, key masking, sigmoid gate:
  t = x @ w_proj.T; per head: q,k,v
  a = softmax(scale*q@k.T + bias + mask); y = a@v
  y = sigmoid(x@w_g.T + b_g) * y;  out = y @ w_o.T + b_o

Sharding: tensor-parallel over heads, 2 heads per core on 8 cores.
Each core runs a fully independent program (no collectives): it computes
its 2 heads' attention plus its 128-column slice of the gate, and a
partial o_proj (contribution of its 128 y-columns to all 1024 outputs).
The host sums the 8 partial outputs and adds b_o (the "all-reduce").

On-device layout is transposed ("scores.T" flash style):
  scores.T[k,q] accumulated in PSUM as  ident@biasT (bias pre-masked,
  pre-transposed on host) + kT.T@qT ; exp on ScalarE (no max-subtraction:
  logits are ~N(0,2) by construction, |logit| < ~14 so exp is safe);
  y.T ext = [v | ones].T @ p gives y.T rows 0..63 and the softmax
  denominator in row 64. Normalization multiplies by a broadcast
  reciprocal (DMA DRAM round-trip to cross partitions).
All matmuls run in float32r (full-rate fp32, ~1.5e-4 relative rounding).

Perf notes (from NTFF traces): every f32r matmul pays a serialized
~LDWEIGHTS+MATMUL pair (~426 ns warm); HAM re-warm never triggers in
this instruction mix, so the kernel must never let the PE idle >3.4us:
one PSUM pool layout for all phases (no pool-transition barrier),
proj-critical DMAs dispatched first, bias stream on the (otherwise
idle) GpSimd DGE queue, and a per-q-tile tail so o_proj/output DMA
overlap the end of attention.
"""
import sys
import numpy as np

try:
    import concourse.bass as bass
except ImportError:
    sys.path.insert(0, "/opt/trn_rl_repo")
    import concourse.bass as bass

import concourse.tile as tile
from concourse import bacc, mybir
from concourse.bass_utils import run_bass_kernel_spmd

B, L, E, H = 1, 2048, 1024, 16
HW = E // H                # 64
SCALE = HW ** -0.5
N_CORES = 8
HPC = H // N_CORES         # 2 heads per core
C2 = HPC * HW              # 128 y-columns per core
MASK_NEG = -60.0           # exp(-60 + max_bias) ~ 1e-23: dead keys vanish

f32 = mybir.dt.float32
f32r = mybir.dt.float32r

NE = E // 128              # 8 contraction chunks
NQ = L // 512              # 4 q-tiles of 512
NKT = L // 128             # 16 k-chunks of 128

_compiled = [None]
DEBUG = False


def _build():
    nc = bacc.Bacc("TRN2", target_bir_lowering=False, debug=False,
                   num_devices=N_CORES)

    xT_ap = nc.dram_tensor("xT", [E, L], f32r, kind="ExternalInput").ap()
    wpT_ap = nc.dram_tensor("wpT", [E, 3 * C2], f32r, kind="ExternalInput").ap()
    biasT_ap = nc.dram_tensor("biasT", [HPC, L, L], f32r, kind="ExternalInput").ap()
    wgT_ap = nc.dram_tensor("wgT", [E, C2], f32r, kind="ExternalInput").ap()
    bgv_ap = nc.dram_tensor("bgv", [C2, 1], f32, kind="ExternalInput").ap()
    woT_ap = nc.dram_tensor("woT", [C2, E], f32r, kind="ExternalInput").ap()
    ident_ap = nc.dram_tensor("ident", [128, 128], f32r, kind="ExternalInput").ap()
    onescols_ap = nc.dram_tensor("onescols", [128, NKT], f32r, kind="ExternalInput").ap()
    outT_ap = nc.dram_tensor("outT", [E, L], f32, kind="ExternalOutput").ap()

    with tile.TileContext(nc) as tc:
        from contextlib import ExitStack
        with ExitStack() as ctx:
            pers = ctx.enter_context(tc.tile_pool(name="pers", bufs=1))
            work = ctx.enter_context(tc.tile_pool(name="work", bufs=1))
            biasp = ctx.enter_context(tc.tile_pool(name="bias", bufs=2))
            sinp = ctx.enter_context(tc.tile_pool(name="sin", bufs=2))
            pp = ctx.enter_context(tc.tile_pool(name="pp", bufs=3))
            nrm = ctx.enter_context(tc.tile_pool(name="nrm", bufs=1))
            dramp = ctx.enter_context(tc.tile_pool(name="dram", bufs=4, space="DRAM"))
            outp = ctx.enter_context(tc.tile_pool(name="outp", bufs=3))
            # one PSUM layout for the whole kernel: no pool-transition barrier
            sp = ctx.enter_context(tc.tile_pool(name="s", bufs=2, space="PSUM"))
            yp = ctx.enter_context(tc.tile_pool(name="y", bufs=1, space="PSUM"))

            # --- proj-critical DMAs first (dispatch order matters) ---
            xT_sb = [pers.tile([128, L], f32r, name=f"xT{e}", tag=f"xT{e}")
                     for e in range(NE)]
            nc.sync.dma_start(xT_sb[0], xT_ap[0:128, :])
            wpT_sb = pers.tile([128, NE, 3 * C2], f32r, tag="wpT")
            nc.sync.dma_start(wpT_sb, wpT_ap.rearrange("(c p) m -> p c m", p=128))
            for e in range(1, NE):
                nc.sync.dma_start(xT_sb[e], xT_ap[e * 128:(e + 1) * 128, :])
            ident_sb = pers.tile([128, 128], f32r, tag="ident")
            nc.sync.dma_start(ident_sb, ident_ap)
            wgT_sb = pers.tile([128, NE, C2], f32r, tag="wgT")
            nc.sync.dma_start(wgT_sb, wgT_ap.rearrange("(c p) m -> p c m", p=128))
            bgv_sb = pers.tile([C2, 1], f32, tag="bgv")
            nc.sync.dma_start(bgv_sb, bgv_ap)
            woT_sb = pers.tile([C2, E], f32r, tag="woT")
            nc.sync.dma_start(woT_sb, woT_ap)
            # v tiles: [128 l, 130] per k-chunk: [v_h0 | ones | v_h1 | ones]
            v_all = pers.tile([128, NKT, 130], f32r, tag="v_all")
            nc.sync.dma_start(v_all[:, :, 64:65], onescols_ap.unsqueeze(2))
            nc.sync.dma_start(v_all[:, :, 129:130], onescols_ap.unsqueeze(2))

            q01 = pers.tile([128, L], f32r, tag="q01")
            k01 = pers.tile([128, L], f32r, tag="k01")
            g_sb = pers.tile([128, L], f32r, tag="g")
            ygT = pers.tile([128, L], f32r, tag="ygT")

            # --- bias stream on GpSimd DGE (keeps Sync queue clear) ---
            bias_tiles = []
            for h in range(HPC):
                for kt in range(NKT):
                    bt = biasp.tile([128, L], f32r, name=f"bias{h}_{kt}", tag="bias")
                    nc.gpsimd.dma_start(
                        bt, biasT_ap[h, kt * 128:(kt + 1) * 128, :])
                    bias_tiles.append(bt)

            # ---------------- proj ----------------
            vT01 = work.tile([128, L], f32r, tag="vT01")
            dests = [q01, k01, vT01]
            for f in range(3):
                for lt in range(NQ):
                    ps = sp.tile([128, 1024], f32, name=f"pj{f}_{lt}", tag="s")
                    for e in range(NE):
                        nc.tensor.matmul(
                            ps[:, 0:512],
                            wpT_sb[:, e, f * 128:(f + 1) * 128],
                            xT_sb[e][:, lt * 512:(lt + 1) * 512],
                            start=(e == 0), stop=(e == NE - 1))
                    nc.vector.tensor_copy(
                        dests[f][:, lt * 512:(lt + 1) * 512], ps[:, 0:512])

            # transpose vT01 -> v_all[:, kt, :]
            for kt in range(NKT):
                ps = sp.tile([128, 128], f32r, name=f"tr{kt}", tag="s")
                nc.tensor.transpose(
                    ps, vT01[:, kt * 128:(kt + 1) * 128], ident_sb)
                nc.vector.tensor_copy(v_all[:, kt, 0:64], ps[:, 0:64])
                nc.vector.tensor_copy(v_all[:, kt, 65:129], ps[:, 64:128])

            # gate: g = sigmoid(wgT.T @ xT + bg)
            for lt in range(NQ):
                ps = sp.tile([C2, 1024], f32, name=f"pg{lt}", tag="s")
                for e in range(NE):
                    nc.tensor.matmul(
                        ps[:, 0:512], wgT_sb[:, e, :],
                        xT_sb[e][:, lt * 512:(lt + 1) * 512],
                        start=(e == 0), stop=(e == NE - 1))
                nc.scalar.activation(
                    g_sb[:, lt * 512:(lt + 1) * 512], ps[:, 0:512],
                    mybir.ActivationFunctionType.Sigmoid,
                    bias=bgv_sb, scale=1.0)

            if DEBUG:
                dbg_q01_ap = nc.dram_tensor("dbg_q01", [128, L], f32, kind="ExternalOutput").ap()
                dbg_sums_ap = nc.dram_tensor("dbg_sums", [HPC, L], f32, kind="ExternalOutput").ap()
                nc.sync.dma_start(dbg_q01_ap, q01.bitcast(f32))

            # ---------------- attention ----------------
            for h in range(HPC):
                hb = h * 64
                y_ps = [yp.tile([65, 512], f32, name=f"y{h}_{i}", tag=f"yq{i}")
                        for i in range(NQ)]
                for kt in range(NKT):
                    bias_t = bias_tiles[h * NKT + kt]
                    for half in range(2):
                        s_ps = sp.tile([128, 1024], f32,
                                       name=f"s{h}_{kt}_{half}", tag="s")
                        for qq in range(2):
                            qs = half * 1024 + qq * 512
                            nc.tensor.matmul(
                                s_ps[:, qq * 512:(qq + 1) * 512],
                                k01[hb:hb + 64, kt * 128:(kt + 1) * 128],
                                q01[hb:hb + 64, qs:qs + 512],
                                start=True, stop=True)
                        s_in = sinp.tile([128, 1024], f32,
                                         name=f"sin{h}_{kt}_{half}", tag="sin")
                        nc.vector.tensor_add(
                            s_in, s_ps,
                            bias_t[:, half * 1024:(half + 1) * 1024])
                        p_t = pp.tile([128, 1024], f32r,
                                      name=f"p{h}_{kt}_{half}", tag="p")
                        nc.scalar.activation(
                            p_t, s_in, mybir.ActivationFunctionType.Exp)
                        for qq in range(2):
                            nc.tensor.matmul(
                                y_ps[half * 2 + qq],
                                v_all[:, kt, h * 65:(h + 1) * 65],
                                p_t[:, qq * 512:(qq + 1) * 512],
                                start=(kt == 0), stop=(kt == NKT - 1))
                # normalization (per q-tile chains): denominators in row 64
                for qt in range(NQ):
                    qsl = slice(qt * 512, (qt + 1) * 512)
                    sums_sb = nrm.tile([65, 512], f32,
                                       name=f"sums{h}_{qt}", tag="sums")
                    nc.vector.tensor_copy(sums_sb[64:65, :], y_ps[qt][64:65, :])
                    if DEBUG:
                        nc.sync.dma_start(dbg_sums_ap[h:h+1, qsl], sums_sb[64:65, :])
                    dscr = dramp.tile([1, 512], f32, name=f"dscr{h}_{qt}", tag="dscr")
                    nc.sync.dma_start(dscr, sums_sb[64:65, :])
                    sums_b = nrm.tile([64, 512], f32,
                                      name=f"sums_b{h}_{qt}", tag="sums_b")
                    nc.sync.dma_start(sums_b, dscr.partition_broadcast(64))
                    rb_sb = nrm.tile([64, 512], f32, name=f"rb{h}_{qt}", tag="rb")
                    nc.vector.reciprocal_approx_fast(rb_sb, sums_b)
                    if h == 0:
                        nc.vector.tensor_mul(
                            ygT[0:64, qsl], y_ps[qt][0:64, :], rb_sb)
                    else:
                        yg1 = nrm.tile([64, 512], f32r,
                                       name=f"yg1_{qt}", tag="yg1")
                        nc.vector.tensor_mul(yg1, y_ps[qt][0:64, :], rb_sb)
                        nc.sync.dma_start(ygT[64:128, qsl], yg1)

            # ---------------- tail: gate mul + o_proj, per q-tile ----------------
            for qt in range(NQ):
                qsl = slice(qt * 512, (qt + 1) * 512)
                nc.vector.tensor_mul(ygT[:, qsl], ygT[:, qsl], g_sb[:, qsl])
                for eo in range(NE):
                    ps = sp.tile([128, 1024], f32, name=f"po{qt}_{eo}", tag="s")
                    nc.tensor.matmul(
                        ps[:, 0:512], woT_sb[:, eo * 128:(eo + 1) * 128],
                        ygT[:, qsl], start=True, stop=True)
                    ot = outp.tile([128, 512], f32, name=f"ot{qt}_{eo}", tag="ot")
                    nc.vector.tensor_copy(ot, ps[:, 0:512])
                    nc.sync.dma_start(
                        outT_ap[eo * 128:(eo + 1) * 128, qsl], ot)

    nc.compile()
    return nc


def kernel(x, mask, bias, w_proj, w_o, b_o, w_g, b_g):
    x = np.asarray(x, dtype=np.float32)
    mask = np.asarray(mask)
    bias = np.asarray(bias, dtype=np.float32)
    w_proj = np.asarray(w_proj, dtype=np.float32)
    w_o = np.asarray(w_o, dtype=np.float32)
    b_o = np.asarray(b_o, dtype=np.float32)
    w_g = np.asarray(w_g, dtype=np.float32)
    b_g = np.asarray(b_g, dtype=np.float32)

    if _compiled[0] is None:
        _compiled[0] = _build()
    nc = _compiled[0]

    xT = np.ascontiguousarray(x[0].T)                      # [E, L]
    mask_add = np.where(mask[0], 0.0, MASK_NEG).astype(np.float32)  # [L]
    ident = np.eye(128, dtype=np.float32)
    onescols = np.ones((128, NKT), dtype=np.float32)

    in_maps = []
    for c in range(N_CORES):
        heads = [c * HPC + i for i in range(HPC)]
        wpT = np.empty((E, 3 * C2), dtype=np.float32)
        for i, h in enumerate(heads):
            r0 = h * 3 * HW
            wpT[:, 0 * C2 + i * HW: 0 * C2 + (i + 1) * HW] = \
                w_proj[r0: r0 + HW].T * SCALE               # q, pre-scaled
            wpT[:, 1 * C2 + i * HW: 1 * C2 + (i + 1) * HW] = \
                w_proj[r0 + HW: r0 + 2 * HW].T              # k
            wpT[:, 2 * C2 + i * HW: 2 * C2 + (i + 1) * HW] = \
                w_proj[r0 + 2 * HW: r0 + 3 * HW].T          # v
        biasT = np.ascontiguousarray(
            bias[0, :, :, heads].transpose(0, 2, 1))        # [2, Lk, Lq]
        biasT += mask_add[None, :, None]
        cols = slice(c * C2, (c + 1) * C2)
        wgT = np.ascontiguousarray(w_g[cols, :].T)          # [E, C2]
        bgv = np.ascontiguousarray(b_g[cols, None])         # [C2, 1]
        woT = np.ascontiguousarray(w_o[:, cols].T)          # [C2, E]
        in_maps.append({
            "xT": xT, "wpT": wpT, "biasT": biasT, "wgT": wgT,
            "bgv": bgv, "woT": woT, "ident": ident, "onescols": onescols,
        })

    res = run_bass_kernel_spmd(nc, in_maps, list(range(N_CORES)))
    acc = res.results[0]["outT"].astype(np.float64)
    for c in range(1, N_CORES):
        acc += res.results[c]["outT"]
    out = acc.T.astype(np.float32) + b_o[None, :]
    return out[None]  # [B, L, E]


# revision 14
# speedup vs baseline: 1.0243x; 1.0243x over previous
"""Trainium2 Bass kernel for nn_Attention_79645873537262.

Dense attention with per-head bias, key masking, sigmoid gate:
  t = x @ w_proj.T; per head: q,k,v
  a = softmax(scale*q@k.T + bias + mask); y = a@v
  y = sigmoid(x@w_g.T + b_g) * y;  out = y @ w_o.T + b_o

Sharding: tensor-parallel over heads, 2 heads per core on 8 cores.
Each core runs a fully independent program (no collectives): it computes
its 2 heads' attention plus its 128-column slice of the gate, and a
partial o_proj (contribution of its 128 y-columns to all 1024 outputs).
The host sums the 8 partial outputs and adds b_o (the "all-reduce").

On-device layout is transposed ("scores.T" flash style):
  scores.T[k,q] accumulated in PSUM as  ident@biasT (bias pre-masked,
  pre-transposed on host) + kT.T@qT ; exp on ScalarE (no max-subtraction:
  logits are ~N(0,2) by construction, |logit| < ~14 so exp is safe);
  y.T ext = [v | ones].T @ p gives y.T rows 0..63 and the softmax
  denominator in row 64. Normalization multiplies by a broadcast
  reciprocal (DMA DRAM round-trip to cross partitions).
All matmuls run in float32r (full-rate fp32, ~1.5e-4 relative rounding).

Perf notes (from NTFF traces): every f32r matmul pays a serialized
~LDWEIGHTS+MATMUL pair (~426 ns warm); HAM re-warm never triggers in
this instruction mix, so the kernel must never let the PE idle >3.4us:
one PSUM pool layout for all phases (no pool-transition barrier),
proj-critical DMAs dispatched first, bias stream on the (otherwise
idle) GpSimd DGE queue, and a per-q-tile tail so o_proj/output DMA
overlap the end of attention.
"""
import sys
import numpy as np

try:
    import concourse.bass as bass
except ImportError:
    sys.path.insert(0, "/opt/trn_rl_repo")
    import concourse.bass as bass

import concourse.tile as tile
from concourse import bacc, mybir
from concourse.bass_utils import run_bass_kernel_spmd

B, L, E, H = 1, 2048, 1024, 16
HW = E // H                # 64
SCALE = HW ** -0.5
N_CORES = 8
HPC = H // N_CORES         # 2 heads per core
C2 = HPC * HW              # 128 y-columns per core
MASK_NEG = -60.0           # exp(-60 + max_bias) ~ 1e-23: dead keys vanish

f32 = mybir.dt.float32
f32r = mybir.dt.float32r

NE = E // 128              # 8 contraction chunks
NQ = L // 512              # 4 q-tiles of 512
NKT = L // 128             # 16 k-chunks of 128

_compiled = [None]
DEBUG = False


def _build():
    nc = bacc.Bacc("TRN2", target_bir_lowering=False, debug=False,
                   num_devices=N_CORES)

    xT_ap = nc.dram_tensor("xT", [E, L], f32r, kind="ExternalInput").ap()
    wpT_ap = nc.dram_tensor("wpT", [E, 3 * C2], f32r, kind="ExternalInput").ap()
    biasT_ap = nc.dram_tensor("biasT", [HPC, L, L], f32r, kind="ExternalInput").ap()
    wgT_ap = nc.dram_tensor("wgT", [E, C2], f32r, kind="ExternalInput").ap()
    bgv_ap = nc.dram_tensor("bgv", [C2, 1], f32, kind="ExternalInput").ap()
    woT_ap = nc.dram_tensor("woT", [C2, E], f32r, kind="ExternalInput").ap()
    ident_ap = nc.dram_tensor("ident", [128, 128], f32r, kind="ExternalInput").ap()
    onescols_ap = nc.dram_tensor("onescols", [128, NKT], f32r, kind="ExternalInput").ap()
    outT_ap = nc.dram_tensor("outT", [E, L], f32, kind="ExternalOutput").ap()

    with tile.TileContext(nc) as tc:
        from contextlib import ExitStack
        with ExitStack() as ctx:
            pers = ctx.enter_context(tc.tile_pool(name="pers", bufs=1))
            work = ctx.enter_context(tc.tile_pool(name="work", bufs=1))
            biasp = ctx.enter_context(tc.tile_pool(name="bias", bufs=3))
            pp = ctx.enter_context(tc.tile_pool(name="pp", bufs=3))
            nrm = ctx.enter_context(tc.tile_pool(name="nrm", bufs=1))
            dramp = ctx.enter_context(tc.tile_pool(name="dram", bufs=4, space="DRAM"))
            outp = ctx.enter_context(tc.tile_pool(name="outp", bufs=3))
            # one PSUM layout for the whole kernel: no pool-transition barrier
            sp = ctx.enter_context(tc.tile_pool(name="s", bufs=2, space="PSUM"))
            yp = ctx.enter_context(tc.tile_pool(name="y", bufs=1, space="PSUM"))

            # --- proj-critical DMAs first (dispatch order matters) ---
            xT_sb = [pers.tile([128, L], f32r, name=f"xT{e}", tag=f"xT{e}")
                     for e in range(NE)]
            nc.sync.dma_start(xT_sb[0], xT_ap[0:128, :])
            wpT_sb = pers.tile([128, NE, 3 * C2], f32r, tag="wpT")
            nc.sync.dma_start(wpT_sb, wpT_ap.rearrange("(c p) m -> p c m", p=128))
            for e in range(1, NE):
                nc.sync.dma_start(xT_sb[e], xT_ap[e * 128:(e + 1) * 128, :])
            ident_sb = pers.tile([128, 128], f32r, tag="ident")
            nc.sync.dma_start(ident_sb, ident_ap)
            wgT_sb = pers.tile([128, NE, C2], f32r, tag="wgT")
            nc.sync.dma_start(wgT_sb, wgT_ap.rearrange("(c p) m -> p c m", p=128))
            bgv_sb = pers.tile([C2, 1], f32, tag="bgv")
            nc.sync.dma_start(bgv_sb, bgv_ap)
            woT_sb = pers.tile([C2, E], f32r, tag="woT")
            nc.sync.dma_start(woT_sb, woT_ap)
            # v tiles: [128 l, 130] per k-chunk: [v_h0 | ones | v_h1 | ones]
            v_all = pers.tile([128, NKT, 130], f32r, tag="v_all")
            nc.sync.dma_start(v_all[:, :, 64:65], onescols_ap.unsqueeze(2))
            nc.sync.dma_start(v_all[:, :, 129:130], onescols_ap.unsqueeze(2))

            q01 = pers.tile([128, L], f32r, tag="q01")
            k01 = pers.tile([128, L], f32r, tag="k01")
            g_sb = pers.tile([128, L], f32r, tag="g")
            ygT = pers.tile([128, L], f32r, tag="ygT")

            # --- bias stream on GpSimd DGE (keeps Sync queue clear) ---
            bias_tiles = []
            for h in range(HPC):
                for kt in range(NKT):
                    bt = biasp.tile([128, L], f32r, name=f"bias{h}_{kt}", tag="bias")
                    nc.gpsimd.dma_start(
                        bt, biasT_ap[h, kt * 128:(kt + 1) * 128, :])
                    bias_tiles.append(bt)

            # ---------------- proj ----------------
            vT01 = work.tile([128, L], f32r, tag="vT01")
            dests = [q01, k01, vT01]
            for f in range(3):
                for lt in range(NQ):
                    ps = sp.tile([128, 1024], f32, name=f"pj{f}_{lt}", tag="s")
                    for e in range(NE):
                        nc.tensor.matmul(
                            ps[:, 0:512],
                            wpT_sb[:, e, f * 128:(f + 1) * 128],
                            xT_sb[e][:, lt * 512:(lt + 1) * 512],
                            start=(e == 0), stop=(e == NE - 1))
                    nc.vector.tensor_copy(
                        dests[f][:, lt * 512:(lt + 1) * 512], ps[:, 0:512])

            # transpose vT01 -> v_all[:, kt, :]
            for kt in range(NKT):
                ps = sp.tile([128, 128], f32r, name=f"tr{kt}", tag="s")
                nc.tensor.transpose(
                    ps, vT01[:, kt * 128:(kt + 1) * 128], ident_sb)
                nc.vector.tensor_copy(v_all[:, kt, 0:64], ps[:, 0:64])
                nc.vector.tensor_copy(v_all[:, kt, 65:129], ps[:, 64:128])

            # gate: g = sigmoid(wgT.T @ xT + bg)
            for lt in range(NQ):
                ps = sp.tile([C2, 1024], f32, name=f"pg{lt}", tag="s")
                for e in range(NE):
                    nc.tensor.matmul(
                        ps[:, 0:512], wgT_sb[:, e, :],
                        xT_sb[e][:, lt * 512:(lt + 1) * 512],
                        start=(e == 0), stop=(e == NE - 1))
                nc.scalar.activation(
                    g_sb[:, lt * 512:(lt + 1) * 512], ps[:, 0:512],
                    mybir.ActivationFunctionType.Sigmoid,
                    bias=bgv_sb, scale=1.0)

            if DEBUG:
                dbg_q01_ap = nc.dram_tensor("dbg_q01", [128, L], f32, kind="ExternalOutput").ap()
                dbg_sums_ap = nc.dram_tensor("dbg_sums", [HPC, L], f32, kind="ExternalOutput").ap()
                nc.sync.dma_start(dbg_q01_ap, q01.bitcast(f32))

            # ---------------- attention ----------------
            for h in range(HPC):
                hb = h * 64
                y_ps = [yp.tile([65, 512], f32, name=f"y{h}_{i}", tag=f"yq{i}")
                        for i in range(NQ)]
                for kt in range(NKT):
                    bias_t = bias_tiles[h * NKT + kt]
                    for half in range(2):
                        s_ps = sp.tile([128, 1024], f32,
                                       name=f"s{h}_{kt}_{half}", tag="s")
                        for qq in range(2):
                            qs = half * 1024 + qq * 512
                            nc.tensor.matmul(
                                s_ps[:, qq * 512:(qq + 1) * 512],
                                ident_sb, bias_t[:, qs:qs + 512],
                                start=True, stop=False)
                            nc.tensor.matmul(
                                s_ps[:, qq * 512:(qq + 1) * 512],
                                k01[hb:hb + 64, kt * 128:(kt + 1) * 128],
                                q01[hb:hb + 64, qs:qs + 512],
                                start=False, stop=True)
                        p_t = pp.tile([128, 1024], f32r,
                                      name=f"p{h}_{kt}_{half}", tag="p")
                        nc.scalar.activation(
                            p_t, s_ps, mybir.ActivationFunctionType.Exp)
                        for qq in range(2):
                            nc.tensor.matmul(
                                y_ps[half * 2 + qq],
                                v_all[:, kt, h * 65:(h + 1) * 65],
                                p_t[:, qq * 512:(qq + 1) * 512],
                                start=(kt == 0), stop=(kt == NKT - 1))
                # normalization (per q-tile chains): denominators in row 64
                for qt in range(NQ):
                    qsl = slice(qt * 512, (qt + 1) * 512)
                    sums_sb = nrm.tile([65, 512], f32,
                                       name=f"sums{h}_{qt}", tag="sums")
                    nc.vector.tensor_copy(sums_sb[64:65, :], y_ps[qt][64:65, :])
                    if DEBUG:
                        nc.sync.dma_start(dbg_sums_ap[h:h+1, qsl], sums_sb[64:65, :])
                    dscr = dramp.tile([1, 512], f32, name=f"dscr{h}_{qt}", tag="dscr")
                    nc.sync.dma_start(dscr, sums_sb[64:65, :])
                    sums_b = nrm.tile([64, 512], f32,
                                      name=f"sums_b{h}_{qt}", tag="sums_b")
                    nc.sync.dma_start(sums_b, dscr.partition_broadcast(64))
                    rb_sb = nrm.tile([64, 512], f32, name=f"rb{h}_{qt}", tag="rb")
                    nc.vector.reciprocal_approx_fast(rb_sb, sums_b)
                    if h == 0:
                        nc.vector.tensor_mul(
                            ygT[0:64, qsl], y_ps[qt][0:64, :], rb_sb)
                    else:
                        yg1 = nrm.tile([64, 512], f32r,
                                       name=f"yg1_{qt}", tag="yg1")
                        nc.vector.tensor_mul(yg1, y_ps[qt][0:64, :], rb_sb)
                        nc.sync.dma_start(ygT[64:128, qsl], yg1)

            # ---------------- tail: gate mul + o_proj, per q-tile ----------------
            for qt in range(NQ):
                qsl = slice(qt * 512, (qt + 1) * 512)
                nc.vector.tensor_mul(ygT[:, qsl], ygT[:, qsl], g_sb[:, qsl])
                for eo in range(NE):
                    ps = sp.tile([128, 1024], f32, name=f"po{qt}_{eo}", tag="s")
                    nc.tensor.matmul(
                        ps[:, 0:512], woT_sb[:, eo * 128:(eo + 1) * 128],
                        ygT[:, qsl], start=True, stop=True)
                    ot = outp.tile([128, 512], f32, name=f"ot{qt}_{eo}", tag="ot")
                    nc.scalar.copy(ot, ps[:, 0:512])
                    nc.sync.dma_start(
                        outT_ap[eo * 128:(eo + 1) * 128, qsl], ot)

    nc.compile()
    return nc


def kernel(x, mask, bias, w_proj, w_o, b_o, w_g, b_g):
    x = np.asarray(x, dtype=np.float32)
    mask = np.asarray(mask)
    bias = np.asarray(bias, dtype=np.float32)
    w_proj = np.asarray(w_proj, dtype=np.float32)
    w_o = np.asarray(w_o, dtype=np.float32)
    b_o = np.asarray(b_o, dtype=np.float32)
    w_g = np.asarray(w_g, dtype=np.float32)
    b_g = np.asarray(b_g, dtype=np.float32)

    if _compiled[0] is None:
        _compiled[0] = _build()
    nc = _compiled[0]

    xT = np.ascontiguousarray(x[0].T)                      # [E, L]
    mask_add = np.where(mask[0], 0.0, MASK_NEG).astype(np.float32)  # [L]
    ident = np.eye(128, dtype=np.float32)
    onescols = np.ones((128, NKT), dtype=np.float32)

    in_maps = []
    for c in range(N_CORES):
        heads = [c * HPC + i for i in range(HPC)]
        wpT = np.empty((E, 3 * C2), dtype=np.float32)
        for i, h in enumerate(heads):
            r0 = h * 3 * HW
            wpT[:, 0 * C2 + i * HW: 0 * C2 + (i + 1) * HW] = \
                w_proj[r0: r0 + HW].T * SCALE               # q, pre-scaled
            wpT[:, 1 * C2 + i * HW: 1 * C2 + (i + 1) * HW] = \
                w_proj[r0 + HW: r0 + 2 * HW].T              # k
            wpT[:, 2 * C2 + i * HW: 2 * C2 + (i + 1) * HW] = \
                w_proj[r0 + 2 * HW: r0 + 3 * HW].T          # v
        biasT = np.ascontiguousarray(
            bias[0, :, :, heads].transpose(0, 2, 1))        # [2, Lk, Lq]
        biasT += mask_add[None, :, None]
        cols = slice(c * C2, (c + 1) * C2)
        wgT = np.ascontiguousarray(w_g[cols, :].T)          # [E, C2]
        bgv = np.ascontiguousarray(b_g[cols, None])         # [C2, 1]
        woT = np.ascontiguousarray(w_o[:, cols].T)          # [C2, E]
        in_maps.append({
            "xT": xT, "wpT": wpT, "biasT": biasT, "wgT": wgT,
            "bgv": bgv, "woT": woT, "ident": ident, "onescols": onescols,
        })

    res = run_bass_kernel_spmd(nc, in_maps, list(range(N_CORES)))
    acc = res.results[0]["outT"].astype(np.float64)
    for c in range(1, N_CORES):
        acc += res.results[c]["outT"]
    out = acc.T.astype(np.float32) + b_o[None, :]
    return out[None]  # [B, L, E]


# revision 15
# speedup vs baseline: 1.0757x; 1.0503x over previous
"""Trainium2 Bass kernel for nn_Attention_79645873537262.

Dense attention with per-head bias, key masking, sigmoid gate:
  t = x @ w_proj.T; per head: q,k,v
  a = softmax(scale*q@k.T + bias + mask); y = a@v
  y = sigmoid(x@w_g.T + b_g) * y;  out = y @ w_o.T + b_o

Sharding: tensor-parallel over heads, 2 heads per core on 8 cores.
Each core runs a fully independent program (no collectives): it computes
its 2 heads' attention plus its 128-column slice of the gate, and a
partial o_proj (contribution of its 128 y-columns to all 1024 outputs).
The host sums the 8 partial outputs and adds b_o (the "all-reduce").

On-device layout is transposed ("scores.T" flash style):
  scores.T[k,q] accumulated in PSUM as  ident@biasT (bias pre-masked,
  pre-transposed on host) + kT.T@qT ; exp on ScalarE (no max-subtraction:
  logits are ~N(0,2) by construction, |logit| < ~14 so exp is safe);
  y.T ext = [v | ones].T @ p gives y.T rows 0..63 and the softmax
  denominator in row 64. Normalization multiplies by a broadcast
  reciprocal (DMA DRAM round-trip to cross partitions).
All matmuls run in float32r (full-rate fp32, ~1.5e-4 relative rounding).

Perf notes (from NTFF traces): every f32r matmul pays a serialized
~LDWEIGHTS+MATMUL pair (~426 ns warm); HAM re-warm never triggers in
this instruction mix, so the kernel must never let the PE idle >3.4us:
one PSUM pool layout for all phases (no pool-transition barrier),
proj-critical DMAs dispatched first, bias stream on the (otherwise
idle) GpSimd DGE queue, and a per-q-tile tail so o_proj/output DMA
overlap the end of attention.
"""
import sys
import numpy as np

try:
    import concourse.bass as bass
except ImportError:
    sys.path.insert(0, "/opt/trn_rl_repo")
    import concourse.bass as bass

import concourse.tile as tile
from concourse import bacc, mybir
from concourse.bass_utils import run_bass_kernel_spmd

B, L, E, H = 1, 2048, 1024, 16
HW = E // H                # 64
SCALE = HW ** -0.5
N_CORES = 8
HPC = H // N_CORES         # 2 heads per core
C2 = HPC * HW              # 128 y-columns per core
MASK_NEG = -60.0           # exp(-60 + max_bias) ~ 1e-23: dead keys vanish

f32 = mybir.dt.float32
f32r = mybir.dt.float32r

NE = E // 128              # 8 contraction chunks
NQ = L // 512              # 4 q-tiles of 512
NKT = L // 128             # 16 k-chunks of 128

_compiled = [None]
DEBUG = False


def _build():
    nc = bacc.Bacc("TRN2", target_bir_lowering=False, debug=False,
                   num_devices=N_CORES)

    xT_ap = nc.dram_tensor("xT", [E, L], f32r, kind="ExternalInput").ap()
    wpT_ap = nc.dram_tensor("wpT", [E, 3 * C2], f32r, kind="ExternalInput").ap()
    biasT_ap = nc.dram_tensor("biasT", [HPC, L, L], f32r, kind="ExternalInput").ap()
    wgT_ap = nc.dram_tensor("wgT", [E, C2], f32r, kind="ExternalInput").ap()
    bgv_ap = nc.dram_tensor("bgv", [C2, 1], f32, kind="ExternalInput").ap()
    woT_ap = nc.dram_tensor("woT", [C2, E], f32r, kind="ExternalInput").ap()
    ident_ap = nc.dram_tensor("ident", [128, 128], f32r, kind="ExternalInput").ap()
    onescols_ap = nc.dram_tensor("onescols", [128, NKT], f32r, kind="ExternalInput").ap()
    outT_ap = nc.dram_tensor("outT", [E, L], f32, kind="ExternalOutput").ap()

    with tile.TileContext(nc) as tc:
        from contextlib import ExitStack
        with ExitStack() as ctx:
            pers = ctx.enter_context(tc.tile_pool(name="pers", bufs=1))
            work = ctx.enter_context(tc.tile_pool(name="work", bufs=1))
            biasp = ctx.enter_context(tc.tile_pool(name="bias", bufs=4))
            pp = ctx.enter_context(tc.tile_pool(name="pp", bufs=3))
            nrm = ctx.enter_context(tc.tile_pool(name="nrm", bufs=1))
            dramp = ctx.enter_context(tc.tile_pool(name="dram", bufs=4, space="DRAM"))
            outp = ctx.enter_context(tc.tile_pool(name="outp", bufs=3))
            # one PSUM layout for the whole kernel: no pool-transition barrier
            sp = ctx.enter_context(tc.tile_pool(name="s", bufs=2, space="PSUM"))
            yp = ctx.enter_context(tc.tile_pool(name="y", bufs=1, space="PSUM"))

            # --- proj-critical DMAs first (dispatch order matters) ---
            xT_sb = [pers.tile([128, L], f32r, name=f"xT{e}", tag=f"xT{e}")
                     for e in range(NE)]
            nc.sync.dma_start(xT_sb[0], xT_ap[0:128, :])
            wpT_sb = pers.tile([128, NE, 3 * C2], f32r, tag="wpT")
            nc.sync.dma_start(wpT_sb, wpT_ap.rearrange("(c p) m -> p c m", p=128))
            for e in range(1, NE):
                nc.sync.dma_start(xT_sb[e], xT_ap[e * 128:(e + 1) * 128, :])
            ident_sb = pers.tile([128, 128], f32r, tag="ident")
            nc.sync.dma_start(ident_sb, ident_ap)
            wgT_sb = pers.tile([128, NE, C2], f32r, tag="wgT")
            nc.sync.dma_start(wgT_sb, wgT_ap.rearrange("(c p) m -> p c m", p=128))
            bgv_sb = pers.tile([C2, 1], f32, tag="bgv")
            nc.sync.dma_start(bgv_sb, bgv_ap)
            woT_sb = pers.tile([C2, E], f32r, tag="woT")
            nc.sync.dma_start(woT_sb, woT_ap)
            # v tiles: [128 l, 130] per k-chunk: [v_h0 | ones | v_h1 | ones]
            v_all = pers.tile([128, NKT, 130], f32r, tag="v_all")
            nc.sync.dma_start(v_all[:, :, 64:65], onescols_ap.unsqueeze(2))
            nc.sync.dma_start(v_all[:, :, 129:130], onescols_ap.unsqueeze(2))

            q01 = pers.tile([128, L], f32r, tag="q01")
            k01 = pers.tile([128, L], f32r, tag="k01")
            g_sb = pers.tile([128, L], f32r, tag="g")
            ygT = pers.tile([128, L], f32r, tag="ygT")

            # ---------------- proj ----------------
            vT01 = work.tile([128, L], f32r, tag="vT01")
            dests = [q01, k01, vT01]
            for f in range(3):
                for lt in range(NQ):
                    ps = sp.tile([128, 1024], f32, name=f"pj{f}_{lt}", tag="s")
                    for e in range(NE):
                        nc.tensor.matmul(
                            ps[:, 0:512],
                            wpT_sb[:, e, f * 128:(f + 1) * 128],
                            xT_sb[e][:, lt * 512:(lt + 1) * 512],
                            start=(e == 0), stop=(e == NE - 1))
                    nc.vector.tensor_copy(
                        dests[f][:, lt * 512:(lt + 1) * 512], ps[:, 0:512])

            # transpose vT01 -> v_all[:, kt, :]
            for kt in range(NKT):
                ps = sp.tile([128, 128], f32r, name=f"tr{kt}", tag="s")
                nc.tensor.transpose(
                    ps, vT01[:, kt * 128:(kt + 1) * 128], ident_sb)
                nc.vector.tensor_copy(v_all[:, kt, 0:64], ps[:, 0:64])
                nc.vector.tensor_copy(v_all[:, kt, 65:129], ps[:, 64:128])

            # gate: g = sigmoid(wgT.T @ xT + bg)
            for lt in range(NQ):
                ps = sp.tile([C2, 1024], f32, name=f"pg{lt}", tag="s")
                for e in range(NE):
                    nc.tensor.matmul(
                        ps[:, 0:512], wgT_sb[:, e, :],
                        xT_sb[e][:, lt * 512:(lt + 1) * 512],
                        start=(e == 0), stop=(e == NE - 1))
                nc.scalar.activation(
                    g_sb[:, lt * 512:(lt + 1) * 512], ps[:, 0:512],
                    mybir.ActivationFunctionType.Sigmoid,
                    bias=bgv_sb, scale=1.0)

            # ---------------- attention: 4 passes over (q-half, head) ----------------
            # y psum double-buffered across passes so pass p+1 accumulates
            # while pass p drains through its normalization chain.
            for qhalf in range(2):
                for h in range(HPC):
                    hb = h * 64
                    y_ps = [yp.tile([65, 512], f32, name=f"y{qhalf}_{h}_{i}",
                                    tag=f"y{i}", bufs=2) for i in range(2)]
                    for kt in range(NKT):
                        bias_t = biasp.tile([128, 1024], f32r,
                                            name=f"bias{qhalf}_{h}_{kt}", tag="bias")
                        nc.gpsimd.dma_start(
                            bias_t, biasT_ap[h, kt * 128:(kt + 1) * 128,
                                             qhalf * 1024:(qhalf + 1) * 1024])
                        s_ps = sp.tile([128, 1024], f32,
                                       name=f"s{qhalf}_{h}_{kt}", tag="s")
                        for qq in range(2):
                            qs = qhalf * 1024 + qq * 512
                            nc.tensor.matmul(
                                s_ps[:, qq * 512:(qq + 1) * 512],
                                ident_sb, bias_t[:, qq * 512:(qq + 1) * 512],
                                start=True, stop=False)
                            nc.tensor.matmul(
                                s_ps[:, qq * 512:(qq + 1) * 512],
                                k01[hb:hb + 64, kt * 128:(kt + 1) * 128],
                                q01[hb:hb + 64, qs:qs + 512],
                                start=False, stop=True)
                        p_t = pp.tile([128, 1024], f32r,
                                      name=f"p{qhalf}_{h}_{kt}", tag="p")
                        nc.scalar.activation(
                            p_t, s_ps, mybir.ActivationFunctionType.Exp)
                        for qq in range(2):
                            nc.tensor.matmul(
                                y_ps[qq],
                                v_all[:, kt, h * 65:(h + 1) * 65],
                                p_t[:, qq * 512:(qq + 1) * 512],
                                start=(kt == 0), stop=(kt == NKT - 1))
                    # normalization chains (softmax denominators in row 64)
                    for qq in range(2):
                        qt = qhalf * 2 + qq
                        qsl = slice(qt * 512, (qt + 1) * 512)
                        sums_sb = nrm.tile([65, 512], f32,
                                           name=f"sums{qhalf}_{h}_{qq}", tag="sums")
                        nc.vector.tensor_copy(sums_sb[64:65, :],
                                              y_ps[qq][64:65, :])
                        dscr = dramp.tile([1, 512], f32,
                                          name=f"dscr{qhalf}_{h}_{qq}", tag="dscr")
                        nc.sync.dma_start(dscr, sums_sb[64:65, :])
                        sums_b = nrm.tile([64, 512], f32,
                                          name=f"sums_b{qhalf}_{h}_{qq}", tag="sums_b")
                        nc.sync.dma_start(sums_b, dscr.partition_broadcast(64))
                        rb_sb = nrm.tile([64, 512], f32,
                                         name=f"rb{qhalf}_{h}_{qq}", tag="rb")
                        nc.vector.reciprocal_approx_fast(rb_sb, sums_b)
                        if h == 0:
                            nc.vector.tensor_mul(
                                ygT[0:64, qsl], y_ps[qq][0:64, :], rb_sb)
                        else:
                            yg1 = nrm.tile([64, 512], f32r,
                                           name=f"yg1_{qhalf}_{qq}", tag="yg1")
                            nc.vector.tensor_mul(yg1, y_ps[qq][0:64, :], rb_sb)
                            nc.sync.dma_start(ygT[64:128, qsl], yg1)

                # tail for this q-half (both heads done): gate mul + o_proj
                for qq in range(2):
                    qt = qhalf * 2 + qq
                    qsl = slice(qt * 512, (qt + 1) * 512)
                    nc.vector.tensor_mul(ygT[:, qsl], ygT[:, qsl], g_sb[:, qsl])
                    for eo in range(NE):
                        ps = sp.tile([128, 1024], f32,
                                     name=f"po{qt}_{eo}", tag="s")
                        nc.tensor.matmul(
                            ps[:, 0:512], woT_sb[:, eo * 128:(eo + 1) * 128],
                            ygT[:, qsl], start=True, stop=True)
                        ot = outp.tile([128, 512], f32,
                                       name=f"ot{qt}_{eo}", tag="ot")
                        if eo % 2 == 0:
                            nc.vector.tensor_copy(ot, ps[:, 0:512])
                        else:
                            nc.scalar.copy(ot, ps[:, 0:512])
                        nc.sync.dma_start(
                            outT_ap[eo * 128:(eo + 1) * 128, qsl], ot)

    nc.compile()
    return nc


def kernel(x, mask, bias, w_proj, w_o, b_o, w_g, b_g):
    x = np.asarray(x, dtype=np.float32)
    mask = np.asarray(mask)
    bias = np.asarray(bias, dtype=np.float32)
    w_proj = np.asarray(w_proj, dtype=np.float32)
    w_o = np.asarray(w_o, dtype=np.float32)
    b_o = np.asarray(b_o, dtype=np.float32)
    w_g = np.asarray(w_g, dtype=np.float32)
    b_g = np.asarray(b_g, dtype=np.float32)

    if _compiled[0] is None:
        _compiled[0] = _build()
    nc = _compiled[0]

    xT = np.ascontiguousarray(x[0].T)                      # [E, L]
    mask_add = np.where(mask[0], 0.0, MASK_NEG).astype(np.float32)  # [L]
    ident = np.eye(128, dtype=np.float32)
    onescols = np.ones((128, NKT), dtype=np.float32)

    in_maps = []
    for c in range(N_CORES):
        heads = [c * HPC + i for i in range(HPC)]
        wpT = np.empty((E, 3 * C2), dtype=np.float32)
        for i, h in enumerate(heads):
            r0 = h * 3 * HW
            wpT[:, 0 * C2 + i * HW: 0 * C2 + (i + 1) * HW] = \
                w_proj[r0: r0 + HW].T * SCALE               # q, pre-scaled
            wpT[:, 1 * C2 + i * HW: 1 * C2 + (i + 1) * HW] = \
                w_proj[r0 + HW: r0 + 2 * HW].T              # k
            wpT[:, 2 * C2 + i * HW: 2 * C2 + (i + 1) * HW] = \
                w_proj[r0 + 2 * HW: r0 + 3 * HW].T          # v
        biasT = np.ascontiguousarray(
            bias[0, :, :, heads].transpose(0, 2, 1))        # [2, Lk, Lq]
        biasT += mask_add[None, :, None]
        cols = slice(c * C2, (c + 1) * C2)
        wgT = np.ascontiguousarray(w_g[cols, :].T)          # [E, C2]
        bgv = np.ascontiguousarray(b_g[cols, None])         # [C2, 1]
        woT = np.ascontiguousarray(w_o[:, cols].T)          # [C2, E]
        in_maps.append({
            "xT": xT, "wpT": wpT, "biasT": biasT, "wgT": wgT,
            "bgv": bgv, "woT": woT, "ident": ident, "onescols": onescols,
        })

    res = run_bass_kernel_spmd(nc, in_maps, list(range(N_CORES)))
    acc = res.results[0]["outT"].astype(np.float64)
    for c in range(1, N_CORES):
        acc += res.results[c]["outT"]
    out = acc.T.astype(np.float32) + b_o[None, :]
    return out[None]  # [B, L, E]
